# revision 4
# baseline (speedup 1.0000x reference)
"""Trainium2 Bass kernel for nn_ContextAttention_21457656611319.

Reference math (per batch n):
    xf = x[n] reshaped [C, L], L = H*W = 4096
    q = Wq@xf + bq ; k = Wk@xf + bk ; v = Wv@xf + bv          [C, L]
    S[l,m] = sum_c k[c,l] q[c,m] * (1/sqrt(C))                 [L, L]
    T = softmax(S, axis=m)
    attn[c,m] = sum_l v[c,l] T[l,m]
    out = x + attn

Sharding: 8 cores = 4 batches x 2-way shard of the l (key/value) axis.
Each core computes a partial attn (sum over its l-half); host adds the
two halves plus x.  No collectives.

V2 design (vs the group-flush baseline):
  * exp is split across two engines per 512-wide score chunk:
    chunks {0,2,4,6,7} -> ACT table exp (accum_out gives the row-sum Z
    for free); chunks {1,3,5} -> DVE "Schraudolph" exp: the bf16 bit
    pattern of 2^w is round(128*(w+127-sigma)), so one fused
    tensor_scalar (S*a+b -> uint16, bitcast bf16) computes exp with
    ~3% max error.  The attn term is ~0.026x the residual x, so this
    contributes ~1e-3 relative error to the final output.
    Z for the DVE chunks is one strided 1x TENSOR_REDUCE.
  * attn accumulates in PSUM banks 0-5 (m in [0,3072)) with K=2048
    across all 16 l-tiles - zero mid-loop flushes.  Scores rotate
    through banks 6-7 as [128,512] chunks.  m in [3072,4096) runs as a
    PE-only tail pass (K=2048 into the freed score banks) overlapped
    with the PSUM->SBUF output copies and DMA.
  * output attn partials are written bf16 (host sums in f32).
  * input DMAs: weights/biases first on the sync queue, x chunks on the
    scalar queue in parallel, so projections start ~10us earlier.
"""

import sys

if "/opt/trn_rl_repo" not in sys.path:
    sys.path.insert(0, "/opt/trn_rl_repo")

import numpy as np

N, C, H, W = 4, 128, 64, 64
L = H * W            # 4096
LH = L // 2          # 2048 l-half per core
P = 128              # partitions / l-tile size
NT = LH // P         # 16 l-tiles per core
CK = 512             # score chunk width (1 PSUM bank)
NCK = L // CK        # 8 chunks per tile
MMAIN = 3072         # m-range accumulated in PSUM banks 0-5
NMS = MMAIN // CK    # 6 main attn slices
NCORES = 8
SCALE = float(1.0 / np.sqrt(C))
# Schraudolph constants: bf16(2^w) bits ~= 128*(w + 127 - 0.0430), w = x*log2e
EXPA = float(128.0 * np.log2(np.e) * SCALE)
EXPB = 16256.0 - 5.5 + 0.5   # +0.5: hw f32->u16 convert truncates
ACT_CHUNKS = (0, 2, 4, 6, 7)   # table-exp + accum_out rowsum
DVE_CHUNKS = (1, 3, 5)         # schraudolph; Z via one strided reduce

_CACHE = {}


def _build_nc():
    import concourse.bass as bass
    import concourse.tile as tile
    from concourse import bacc, mybir
    from contextlib import ExitStack

    f32 = mybir.dt.float32
    bf16 = mybir.dt.bfloat16
    u16 = mybir.dt.uint16

    nc = bacc.Bacc("TRN2", target_bir_lowering=False, debug=False)

    xf = nc.dram_tensor("xf", [P, L], bf16, kind="ExternalInput").ap()
    xh = nc.dram_tensor("xh", [P, LH], bf16, kind="ExternalInput").ap()
    wqT = nc.dram_tensor("wqT", [P, P], bf16, kind="ExternalInput").ap()
    wkT = nc.dram_tensor("wkT", [P, P], bf16, kind="ExternalInput").ap()
    wvT = nc.dram_tensor("wvT", [P, P], bf16, kind="ExternalInput").ap()
    bq = nc.dram_tensor("bq", [P, 1], f32, kind="ExternalInput").ap()
    bk = nc.dram_tensor("bk", [P, 1], f32, kind="ExternalInput").ap()
    bv = nc.dram_tensor("bv", [1, P], f32, kind="ExternalInput").ap()
    attn_out = nc.dram_tensor("attn_part", [P, L], bf16,
                              kind="ExternalOutput").ap()

    Exp = mybir.ActivationFunctionType.Exp
    Ident = mybir.ActivationFunctionType.Identity
    Copy = mybir.ActivationFunctionType.Copy
    mult = mybir.AluOpType.mult
    add = mybir.AluOpType.add

    with tile.TileContext(nc) as tc, ExitStack() as ctx:
        const = ctx.enter_context(tc.tile_pool(name="const", bufs=1))
        persist = ctx.enter_context(tc.tile_pool(name="persist", bufs=1))
        sp = ctx.enter_context(tc.tile_pool(name="sps", bufs=2, space="PSUM"))
        app = ctx.enter_context(tc.tile_pool(name="aps", bufs=1, space="PSUM"))

        wq_sb = const.tile([P, P], bf16)
        wk_sb = const.tile([P, P], bf16)
        wv_sb = const.tile([P, P], bf16)
        bq_sb = const.tile([P, 1], f32)
        bk_sb = const.tile([P, 1], f32)
        bv_sb = const.tile([P, P], f32)
        warm = const.tile([P, 1], f32)
        # weights+biases first (they gate the projections); x on the
        # scalar queue in parallel
        nc.sync.dma_start(out=wk_sb, in_=wkT)
        nc.sync.dma_start(out=wv_sb, in_=wvT)
        nc.sync.dma_start(out=wq_sb, in_=wqT)
        nc.sync.dma_start(out=bq_sb, in_=bq)
        nc.sync.dma_start(out=bk_sb, in_=bk)
        bv_bcast = bass.AP(tensor=bv.tensor, offset=bv.offset,
                           ap=[[0, P], bv.ap[1]])
        nc.sync.dma_start(out=bv_sb, in_=bv_bcast)
        # warm the exp table while DMAs land
        nc.scalar.activation(warm, bq_sb, Exp, scale=0.0)

        q_sb = persist.tile([P, L], bf16)
        k_sb = persist.tile([P, LH], bf16)
        vt_sb = persist.tile([P, NT, P], f32)    # [l, tile, c]
        vts = persist.tile([P, NT, P], bf16)     # vT * (1/Z)
        za = persist.tile([P, NT, 8], f32)       # rowsum parts
        zs = persist.tile([P, NT], f32)
        rs = persist.tile([P, NT], f32)
        attn_sb = persist.tile([P, L], bf16)     # output staging

        apsum = app.tile([P, MMAIN], f32)        # banks 0-5

        # ---- projections, streamed through the same PSUM pools ------
        with tc.tile_pool(name="xp", bufs=1) as xp:
            x_sb = xp.tile([P, L], bf16)
            xh_sb = xp.tile([P, LH], bf16)
            nc.sync.dma_start(out=xh_sb[:, :LH // 2], in_=xh[:, :LH // 2])
            nc.sync.dma_start(out=xh_sb[:, LH // 2:], in_=xh[:, LH // 2:])
            for h in range(4):
                msl = slice(h * 1024, (h + 1) * 1024)
                nc.scalar.dma_start(out=x_sb[:, msl], in_=xf[:, msl])

            # vT per l-tile through the (not yet used) attn banks
            for j in range(NT):
                nc.tensor.matmul(apsum[:, j * P:(j + 1) * P],
                                 xh_sb[:, j * P:(j + 1) * P], wv_sb)
            for j in range(NT):
                nc.vector.tensor_add(vt_sb[:, j, :],
                                     apsum[:, j * P:(j + 1) * P], bv_sb)
            for h in range(4):
                t = sp.tile([P, CK], f32, tag="s")
                nc.tensor.matmul(t, wk_sb, xh_sb[:, h * CK:(h + 1) * CK])
                nc.scalar.activation(k_sb[:, h * CK:(h + 1) * CK], t,
                                     Ident, bias=bk_sb)
            for c in range(NCK):
                t = sp.tile([P, CK], f32, tag="s")
                nc.tensor.matmul(t, wq_sb, x_sb[:, c * CK:(c + 1) * CK])
                nc.vector.tensor_scalar_add(q_sb[:, c * CK:(c + 1) * CK],
                                            t, bq_sb)

        # T storage reuses the SBUF freed by xp
        tpool = tc.alloc_tile_pool(name="tpool", bufs=1)
        t_all = tpool.tile([P, NT, L], bf16)

        dve_set = set(DVE_CHUNKS)

        def z_tail(i):
            # combine ACT accums (slots 0-4) + DVE partials (slots 5-7)
            nc.vector.reduce_sum(out=zs[:, i:i + 1], in_=za[:, i, :],
                                 axis=mybir.AxisListType.X)
            nc.vector.reciprocal(rs[:, i:i + 1], zs[:, i:i + 1])
            nc.gpsimd.tensor_scalar_mul(vts[:, i, :], vt_sb[:, i, :],
                                        rs[:, i:i + 1])

        # ---- main loop ----------------------------------------------
        for i in range(NT):
            ai = i - 1
            na = 0  # ACT accum slot
            for c in range(NCK):
                s = sp.tile([P, CK], f32, tag="s")
                m0 = c * CK
                nc.tensor.matmul(s, k_sb[:, i * P:(i + 1) * P],
                                 q_sb[:, m0:m0 + CK])
                if c in dve_set:
                    nc.vector.tensor_scalar(
                        out=t_all[:, i, m0:m0 + CK].bitcast(u16), in0=s,
                        scalar1=EXPA, scalar2=EXPB, op0=mult, op1=add)
                else:
                    nc.scalar.activation(t_all[:, i, m0:m0 + CK], s, Exp,
                                         scale=SCALE,
                                         accum_out=za[:, i, na:na + 1])
                    na += 1
                if c == DVE_CHUNKS[-1]:
                    # Z of the schraudolph chunks: one strided reduce,
                    # overlaps the remaining ACT chunks
                    row = t_all[:, i, :]
                    dview = bass.AP(
                        tensor=row.tensor,
                        offset=row.offset + DVE_CHUNKS[0] * CK,
                        ap=[row.ap[0], [2 * CK, len(DVE_CHUNKS)], [1, CK]])
                    nc.vector.reduce_sum(
                        out=za[:, i, 5:5 + len(DVE_CHUNKS)],
                        in_=dview, axis=mybir.AxisListType.X)
                # attn matmuls for the previous tile fill PE gaps
                if ai >= 0 and 2 <= c:
                    j = c - 2
                    nc.tensor.matmul(
                        apsum[:, j * CK:(j + 1) * CK], vts[:, ai, :],
                        t_all[:, ai, j * CK:(j + 1) * CK],
                        start=(ai == 0), stop=False)
            z_tail(i)

        # ---- tail: tile 15 attn, last m-quarter, output --------------
        ai = NT - 1
        for j in range(NMS):
            nc.tensor.matmul(apsum[:, j * CK:(j + 1) * CK], vts[:, ai, :],
                             t_all[:, ai, j * CK:(j + 1) * CK],
                             start=False, stop=True)
        # pass-B: m in [3072,4096) with K=2048 in the freed score banks
        for jt in range(2):
            m0 = MMAIN + jt * CK
            bt = sp.tile([P, CK], f32, tag="s")
            for i2 in range(NT):
                nc.tensor.matmul(bt, vts[:, i2, :],
                                 t_all[:, i2, m0:m0 + CK],
                                 start=(i2 == 0), stop=(i2 == NT - 1))
            if jt == 0:
                nc.scalar.activation(attn_sb[:, m0:m0 + CK], bt, Copy)
            else:
                nc.vector.tensor_copy(attn_sb[:, m0:m0 + CK], bt)
            nc.sync.dma_start(out=attn_out[:, m0:m0 + CK],
                              in_=attn_sb[:, m0:m0 + CK])
        # drain the main accumulators, interleaved ACT/DVE
        for j in range(NMS):
            msl = slice(j * CK, (j + 1) * CK)
            if j % 2 == 0:
                nc.scalar.activation(attn_sb[:, msl], apsum[:, msl], Copy)
            else:
                nc.vector.tensor_copy(attn_sb[:, msl], apsum[:, msl])
            nc.sync.dma_start(out=attn_out[:, msl], in_=attn_sb[:, msl])
        tpool.release()

    nc.compile()
    return nc


def _get_nc():
    if "nc" not in _CACHE:
        _CACHE["nc"] = _build_nc()
    return _CACHE["nc"]


def _make_in_maps(inputs):
    import ml_dtypes
    bf = ml_dtypes.bfloat16
    x = np.ascontiguousarray(np.asarray(inputs["x"], dtype=np.float32))
    wqT = np.ascontiguousarray(np.asarray(inputs["Wq"], dtype=np.float32).T.astype(bf))
    wkT = np.ascontiguousarray(np.asarray(inputs["Wk"], dtype=np.float32).T.astype(bf))
    wvT = np.ascontiguousarray(np.asarray(inputs["Wv"], dtype=np.float32).T.astype(bf))
    bq = np.ascontiguousarray(np.asarray(inputs["bq"], dtype=np.float32).reshape(P, 1))
    bk = np.ascontiguousarray(np.asarray(inputs["bk"], dtype=np.float32).reshape(P, 1))
    bv = np.ascontiguousarray(np.asarray(inputs["bv"], dtype=np.float32).reshape(1, P))
    in_maps = []
    for core in range(NCORES):
        n, half = core // 2, core % 2
        xf32 = x[n].reshape(C, L)
        xfb = np.ascontiguousarray(xf32.astype(bf))
        xhb = np.ascontiguousarray(xfb[:, half * LH:(half + 1) * LH])
        in_maps.append({
            "xf": xfb, "xh": xhb,
            "wqT": wqT, "wkT": wkT, "wvT": wvT,
            "bq": bq, "bk": bk, "bv": bv,
        })
    return in_maps, x


def run_on_hw(inputs, trace=False, **kwargs):
    from concourse import bass_utils
    nc = _get_nc()
    in_maps, _ = _make_in_maps(inputs)
    res = bass_utils.run_bass_kernel_spmd(
        nc, in_maps, list(range(NCORES)), trace=trace, **kwargs)
    parts = [res.results[i]["attn_part"] for i in range(NCORES)]
    return parts, res


def kernel(**inputs) -> np.ndarray:
    in_maps, x = _make_in_maps(inputs)
    parts, _ = run_on_hw(inputs)
    out = np.empty((N, C, H, W), dtype=np.float32)
    for n in range(N):
        attn = parts[2 * n].astype(np.float32) + parts[2 * n + 1].astype(np.float32)
        out[n] = x[n] + attn.reshape(C, H, W)
    return out


# revision 7
# speedup vs baseline: 1.3131x; 1.3131x over previous
"""Trainium2 Bass kernel for nn_ContextAttention_21457656611319.

Reference math (per batch n):
    xf = x[n] reshaped [C, L], L = H*W = 4096
    q = Wq@xf + bq ; k = Wk@xf + bk ; v = Wv@xf + bv          [C, L]
    S[l,m] = sum_c k[c,l] q[c,m] * (1/sqrt(C))                 [L, L]
    T = softmax(S, axis=m)
    attn[c,m] = sum_l v[c,l] T[l,m]
    out = x + attn

Sharding: 8 cores = 4 batches x 2-way shard of the l (key/value) axis.
Each core computes a partial attn (sum over its l-half); host adds the
two halves plus x.  No collectives.

V2 design (vs the group-flush baseline):
  * exp is split across two engines per 512-wide score chunk:
    chunks {0,2,4,6,7} -> ACT table exp (accum_out gives the row-sum Z
    for free); chunks {1,3,5} -> DVE "Schraudolph" exp: the bf16 bit
    pattern of 2^w is round(128*(w+127-sigma)), so one fused
    tensor_scalar (S*a+b -> uint16, bitcast bf16) computes exp with
    ~3% max error.  The attn term is ~0.026x the residual x, so this
    contributes ~1e-3 relative error to the final output.
    Z for the DVE chunks is one strided 1x TENSOR_REDUCE.
  * attn accumulates in PSUM banks 0-5 (m in [0,3072)) with K=2048
    across all 16 l-tiles - zero mid-loop flushes.  Scores rotate
    through banks 6-7 as [128,512] chunks.  m in [3072,4096) runs as a
    PE-only tail pass (K=2048 into the freed score banks) overlapped
    with the PSUM->SBUF output copies and DMA.
  * output attn partials are written bf16 (host sums in f32).
  * input DMAs: weights/biases first on the sync queue, x chunks on the
    scalar queue in parallel, so projections start ~10us earlier.
"""

import sys

if "/opt/trn_rl_repo" not in sys.path:
    sys.path.insert(0, "/opt/trn_rl_repo")

import numpy as np

N, C, H, W = 4, 128, 64, 64
L = H * W            # 4096
LH = L // 2          # 2048 l-half per core
P = 128              # partitions / l-tile size
NT = LH // P         # 16 l-tiles per core
CK = 512             # score chunk width (1 PSUM bank)
NCK = L // CK        # 8 chunks per tile
MMAIN = 3072         # m-range accumulated in PSUM banks 0-5
NMS = MMAIN // CK    # 6 main attn slices
NCORES = 8
SCALE = float(1.0 / np.sqrt(C))
# Schraudolph constants: bf16(2^w) bits ~= 128*(w + 127 - 0.0430), w = x*log2e
EXPA = float(128.0 * np.log2(np.e) * SCALE)
EXPB = 16256.0 - 5.5 + 0.5   # +0.5: hw f32->u16 convert truncates
ACT_CHUNKS = (0, 2, 4, 6, 7)   # table-exp + accum_out rowsum
DVE_CHUNKS = (1, 3, 5)         # schraudolph; Z via one strided reduce

_CACHE = {}


def _build_nc():
    import concourse.bass as bass
    import concourse.tile as tile
    from concourse import bacc, mybir
    from contextlib import ExitStack

    f32 = mybir.dt.float32
    bf16 = mybir.dt.bfloat16
    u16 = mybir.dt.uint16

    nc = bacc.Bacc("TRN2", target_bir_lowering=False, debug=False)

    xf = nc.dram_tensor("xf", [P, L], bf16, kind="ExternalInput").ap()
    xh = nc.dram_tensor("xh", [P, LH], bf16, kind="ExternalInput").ap()
    wqT = nc.dram_tensor("wqT", [P, P], bf16, kind="ExternalInput").ap()
    wkT = nc.dram_tensor("wkT", [P, P], bf16, kind="ExternalInput").ap()
    wvT = nc.dram_tensor("wvT", [P, P], bf16, kind="ExternalInput").ap()
    bq = nc.dram_tensor("bq", [P, 1], f32, kind="ExternalInput").ap()
    bk = nc.dram_tensor("bk", [P, 1], f32, kind="ExternalInput").ap()
    bv = nc.dram_tensor("bv", [1, P], f32, kind="ExternalInput").ap()
    attn_out = nc.dram_tensor("attn_part", [P, L], bf16,
                              kind="ExternalOutput").ap()

    Exp = mybir.ActivationFunctionType.Exp
    Ident = mybir.ActivationFunctionType.Identity
    Copy = mybir.ActivationFunctionType.Copy
    mult = mybir.AluOpType.mult
    add = mybir.AluOpType.add

    with tile.TileContext(nc) as tc, ExitStack() as ctx:
        const = ctx.enter_context(tc.tile_pool(name="const", bufs=1))
        persist = ctx.enter_context(tc.tile_pool(name="persist", bufs=1))
        sp = ctx.enter_context(tc.tile_pool(name="sps", bufs=2, space="PSUM"))
        app = ctx.enter_context(tc.tile_pool(name="aps", bufs=1, space="PSUM"))

        wq_sb = const.tile([P, P], bf16)
        wk_sb = const.tile([P, P], bf16)
        wv_sb = const.tile([P, P], bf16)
        bq_sb = const.tile([P, 1], f32)
        bk_sb = const.tile([P, 1], f32)
        bv_sb = const.tile([P, P], f32)
        warm = const.tile([P, 1], f32)
        # weights+biases first (they gate the projections); x on the
        # scalar queue in parallel
        nc.sync.dma_start(out=wk_sb, in_=wkT)
        nc.sync.dma_start(out=wv_sb, in_=wvT)
        nc.sync.dma_start(out=wq_sb, in_=wqT)
        nc.sync.dma_start(out=bq_sb, in_=bq)
        nc.sync.dma_start(out=bk_sb, in_=bk)
        bv_bcast = bass.AP(tensor=bv.tensor, offset=bv.offset,
                           ap=[[0, P], bv.ap[1]])
        nc.sync.dma_start(out=bv_sb, in_=bv_bcast)
        # warm the exp table while DMAs land
        nc.scalar.activation(warm, bq_sb, Exp, scale=0.0)

        q_sb = persist.tile([P, L], bf16)
        k_sb = persist.tile([P, LH], bf16)
        vt_sb = persist.tile([P, NT, P], f32)    # [l, tile, c]
        vts = persist.tile([P, NT, P], bf16)     # vT * (1/Z)
        za = persist.tile([P, NT, 8], f32)       # rowsum parts
        zs = persist.tile([P, NT], f32)
        rs = persist.tile([P, NT], f32)
        attn_sb = persist.tile([P, L], bf16)     # output staging

        apsum = app.tile([P, MMAIN], f32)        # banks 0-5

        # ---- projections, streamed through the same PSUM pools ------
        with tc.tile_pool(name="xp", bufs=1) as xp:
            x_sb = xp.tile([P, L], bf16)
            xh_sb = xp.tile([P, LH], bf16)
            nc.sync.dma_start(out=xh_sb[:, :LH // 2], in_=xh[:, :LH // 2])
            nc.sync.dma_start(out=xh_sb[:, LH // 2:], in_=xh[:, LH // 2:])
            for h in range(4):
                msl = slice(h * 1024, (h + 1) * 1024)
                nc.scalar.dma_start(out=x_sb[:, msl], in_=xf[:, msl])

            # vT per l-tile through the (not yet used) attn banks
            for j in range(NT):
                nc.tensor.matmul(apsum[:, j * P:(j + 1) * P],
                                 xh_sb[:, j * P:(j + 1) * P], wv_sb)
            for j in range(NT):
                nc.vector.tensor_add(vt_sb[:, j, :],
                                     apsum[:, j * P:(j + 1) * P], bv_sb)
            for h in range(4):
                t = sp.tile([P, CK], f32, tag="s")
                nc.tensor.matmul(t, wk_sb, xh_sb[:, h * CK:(h + 1) * CK])
                nc.scalar.activation(k_sb[:, h * CK:(h + 1) * CK], t,
                                     Ident, bias=bk_sb)
            for c in range(NCK):
                t = sp.tile([P, CK], f32, tag="s")
                nc.tensor.matmul(t, wq_sb, x_sb[:, c * CK:(c + 1) * CK])
                nc.vector.tensor_scalar_add(q_sb[:, c * CK:(c + 1) * CK],
                                            t, bq_sb)

        # T storage reuses the SBUF freed by xp
        tpool = tc.alloc_tile_pool(name="tpool", bufs=1)
        t_all = tpool.tile([P, NT, L], bf16)

        dve_set = set(DVE_CHUNKS)

        def z_tail(i):
            # combine ACT accums (slots 0-4) + DVE partials (slots 5-7)
            nc.vector.reduce_sum(out=zs[:, i:i + 1], in_=za[:, i, :],
                                 axis=mybir.AxisListType.X)
            nc.vector.reciprocal(rs[:, i:i + 1], zs[:, i:i + 1])
            nc.vector.tensor_scalar_mul(vts[:, i, :], vt_sb[:, i, :],
                                        rs[:, i:i + 1])

        # ---- main loop ----------------------------------------------
        # attn matmuls run two tiles behind their exp so the Z ->
        # reciprocal -> vts chain is never on the in-order PE's
        # critical path (one tile behind stalled PE ~2us per tile).
        for i in range(NT):
            ai = i - 2
            na = 0  # ACT accum slot
            for c in range(NCK):
                s = sp.tile([P, CK], f32, tag="s")
                m0 = c * CK
                nc.tensor.matmul(s, k_sb[:, i * P:(i + 1) * P],
                                 q_sb[:, m0:m0 + CK])
                if c in dve_set:
                    nc.vector.tensor_scalar(
                        out=t_all[:, i, m0:m0 + CK].bitcast(u16), in0=s,
                        scalar1=EXPA, scalar2=EXPB, op0=mult, op1=add)
                else:
                    nc.scalar.activation(t_all[:, i, m0:m0 + CK], s, Exp,
                                         scale=SCALE,
                                         accum_out=za[:, i, na:na + 1])
                    na += 1
                if c == DVE_CHUNKS[-1]:
                    # Z of the schraudolph chunks: one strided reduce,
                    # overlaps the remaining ACT chunks
                    row = t_all[:, i, :]
                    dview = bass.AP(
                        tensor=row.tensor,
                        offset=row.offset + DVE_CHUNKS[0] * CK,
                        ap=[row.ap[0], [2 * CK, len(DVE_CHUNKS)], [1, CK]])
                    nc.vector.reduce_sum(
                        out=za[:, i, 5:5 + len(DVE_CHUNKS)],
                        in_=dview, axis=mybir.AxisListType.X)
                # attn matmuls for tile i-2 fill PE gaps
                if ai >= 0 and c < NMS:
                    nc.tensor.matmul(
                        apsum[:, c * CK:(c + 1) * CK], vts[:, ai, :],
                        t_all[:, ai, c * CK:(c + 1) * CK],
                        start=(ai == 0), stop=False)
            z_tail(i)

        # ---- tail: tiles 14+15 attn, last m-quarter, output ----------
        for ai in (NT - 2, NT - 1):
            for j in range(NMS):
                nc.tensor.matmul(apsum[:, j * CK:(j + 1) * CK],
                                 vts[:, ai, :],
                                 t_all[:, ai, j * CK:(j + 1) * CK],
                                 start=False, stop=(ai == NT - 1))
        # pass-B: m in [3072,4096) with K=2048 in the freed score banks
        for jt in range(2):
            m0 = MMAIN + jt * CK
            bt = sp.tile([P, CK], f32, tag="s")
            for i2 in range(NT):
                nc.tensor.matmul(bt, vts[:, i2, :],
                                 t_all[:, i2, m0:m0 + CK],
                                 start=(i2 == 0), stop=(i2 == NT - 1))
            if jt == 0:
                nc.scalar.activation(attn_sb[:, m0:m0 + CK], bt, Copy)
            else:
                nc.vector.tensor_copy(attn_sb[:, m0:m0 + CK], bt)
            nc.sync.dma_start(out=attn_out[:, m0:m0 + CK],
                              in_=attn_sb[:, m0:m0 + CK])
        # drain the main accumulators, interleaved ACT/DVE
        for j in range(NMS):
            msl = slice(j * CK, (j + 1) * CK)
            if j % 2 == 0:
                nc.scalar.activation(attn_sb[:, msl], apsum[:, msl], Copy)
            else:
                nc.vector.tensor_copy(attn_sb[:, msl], apsum[:, msl])
            nc.sync.dma_start(out=attn_out[:, msl], in_=attn_sb[:, msl])
        tpool.release()

    nc.compile()
    return nc


def _get_nc():
    if "nc" not in _CACHE:
        _CACHE["nc"] = _build_nc()
    return _CACHE["nc"]


def _make_in_maps(inputs):
    import ml_dtypes
    bf = ml_dtypes.bfloat16
    x = np.ascontiguousarray(np.asarray(inputs["x"], dtype=np.float32))
    wqT = np.ascontiguousarray(np.asarray(inputs["Wq"], dtype=np.float32).T.astype(bf))
    wkT = np.ascontiguousarray(np.asarray(inputs["Wk"], dtype=np.float32).T.astype(bf))
    wvT = np.ascontiguousarray(np.asarray(inputs["Wv"], dtype=np.float32).T.astype(bf))
    bq = np.ascontiguousarray(np.asarray(inputs["bq"], dtype=np.float32).reshape(P, 1))
    bk = np.ascontiguousarray(np.asarray(inputs["bk"], dtype=np.float32).reshape(P, 1))
    bv = np.ascontiguousarray(np.asarray(inputs["bv"], dtype=np.float32).reshape(1, P))
    in_maps = []
    for core in range(NCORES):
        n, half = core // 2, core % 2
        xf32 = x[n].reshape(C, L)
        xfb = np.ascontiguousarray(xf32.astype(bf))
        xhb = np.ascontiguousarray(xfb[:, half * LH:(half + 1) * LH])
        in_maps.append({
            "xf": xfb, "xh": xhb,
            "wqT": wqT, "wkT": wkT, "wvT": wvT,
            "bq": bq, "bk": bk, "bv": bv,
        })
    return in_maps, x


def run_on_hw(inputs, trace=False, **kwargs):
    from concourse import bass_utils
    nc = _get_nc()
    in_maps, _ = _make_in_maps(inputs)
    res = bass_utils.run_bass_kernel_spmd(
        nc, in_maps, list(range(NCORES)), trace=trace, **kwargs)
    parts = [res.results[i]["attn_part"] for i in range(NCORES)]
    return parts, res


def kernel(**inputs) -> np.ndarray:
    in_maps, x = _make_in_maps(inputs)
    parts, _ = run_on_hw(inputs)
    out = np.empty((N, C, H, W), dtype=np.float32)
    for n in range(N):
        attn = parts[2 * n].astype(np.float32) + parts[2 * n + 1].astype(np.float32)
        out[n] = x[n] + attn.reshape(C, H, W)
    return out
